# revision 2
# baseline (speedup 1.0000x reference)
"""CodeWiseAttention kernel for Trainium2 (8 NeuronCores, label-dim sharded).

m[b,n,:] = softmax(label_feature[n] @ x[b].T) @ x[b]

Sharding: label rows N=8922 split across 8 cores (1116/core, padded to 1152);
x replicated. Per core, per batch:
  mm1 (fp32r): S^T[l,n] = xT[e,l].T @ labelT[e,n]     (xT via PE transpose)
  exp on ScalarE: expS = exp(S - 30)                   (constant shift; cancels)
  mm2 (fp32r): Uaug^T[e',n] += xa[l,e'].T @ expS^T[l,n]  accumulated over l,
      where xa has a ones column so row 100 of Uaug = Z = sum_l expS.
  out: PE-transpose Uaug^T -> [n, e'], m = U / Z, DMA out.
"""
import numpy as np
from contextlib import ExitStack

import concourse.tile as tile
from concourse import bacc, mybir
from concourse.bass_utils import run_bass_kernel_spmd

F32 = mybir.dt.float32
F32R = mybir.dt.float32r

B, L, E = 8, 2500, 100
LP = 2520          # L padded with zero rows (zero rows add nothing to U or Z)
N_TOTAL = 8922
NCORES = 8
NS = 1116          # label rows per core (core 7: 1110 real)
NSP = 1152         # padded per-core label rows
LC = 126           # l-chunk rows (even: fp32r ISA needs even innermost counts)
NLC = LP // LC     # 20 l-chunks
NCH = 384          # n-chunk width (moving dim; >=256 keeps fp32r at full rate)
NJ = NSP // NCH    # 3 n-chunks
EA = E + 1         # x augmented with ones column
PSB = 512          # psum bank stride in f32 elements
EXP_BIAS = -30.0

TRACE = False
LAST_RESULT = None

_NC = []


def _build():
    nc = bacc.Bacc("TRN2", target_bir_lowering=False, debug=False)
    xa_d = nc.dram_tensor("xa", [B, LP, EA], F32R, kind="ExternalInput").ap()
    lab_d = nc.dram_tensor("lab", [NSP, E], F32R, kind="ExternalInput").ap()
    idr_d = nc.dram_tensor("idr", [128, 128], F32R, kind="ExternalInput").ap()
    idf_d = nc.dram_tensor("idf", [128, 128], F32, kind="ExternalInput").ap()
    m_d = nc.dram_tensor("m", [B, NSP, E], F32, kind="ExternalOutput").ap()

    with tile.TileContext(nc) as tc, ExitStack() as ctx:
        consts = ctx.enter_context(tc.tile_pool(name="consts", bufs=1))
        lab_pool = ctx.enter_context(tc.tile_pool(name="labp", bufs=2))
        xa_pool = ctx.enter_context(tc.tile_pool(name="xap", bufs=2))
        xt_pool = ctx.enter_context(tc.tile_pool(name="xtp", bufs=2))
        e_pool = ctx.enter_context(tc.tile_pool(name="ep", bufs=3))
        u_pool = ctx.enter_context(tc.tile_pool(name="up", bufs=2))
        o_pool = ctx.enter_context(tc.tile_pool(name="op", bufs=4))
        r_pool = ctx.enter_context(tc.tile_pool(name="rp", bufs=4))
        pstr = ctx.enter_context(tc.tile_pool(name="pstr", bufs=2, space="PSUM"))
        pss = ctx.enter_context(tc.tile_pool(name="pss", bufs=1, space="PSUM"))
        psm = ctx.enter_context(tc.tile_pool(name="psm", bufs=1, space="PSUM"))

        idr_sb = consts.tile([128, 128], F32R)
        nc.sync.dma_start(out=idr_sb[:], in_=idr_d)
        idf_sb = consts.tile([128, 128], F32)
        nc.sync.dma_start(out=idf_sb[:], in_=idf_d)
        bias_sb = consts.tile([128, 1], F32)
        nc.vector.memset(bias_sb[:], EXP_BIAS)

        # labelT [E, NSP] via PE transposes of 128-row label chunks
        labT = consts.tile([E, NSP], F32R)
        for k in range(NSP // 128):
            lsb = lab_pool.tile([128, E], F32R, tag="lab")
            nc.sync.dma_start(out=lsb[:], in_=lab_d[k * 128:(k + 1) * 128, :])
            tp = pstr.tile([128, 128], F32R, tag="tr")
            nc.tensor.transpose(tp[:E, :], lsb[:], idr_sb[:, :])
            nc.vector.tensor_copy(labT[:, k * 128:(k + 1) * 128], tp[:E, :])

        for b in range(B):
            xa_sb = xa_pool.tile([LC, NLC, EA], F32R, tag="xa")
            nc.sync.dma_start(
                out=xa_sb[:], in_=xa_d[b].rearrange("(c p) e -> p c e", p=LC)
            )
            # xT [E, L] for this batch
            xT = xt_pool.tile([E, LP], F32R, tag="xt")
            for c in range(NLC):
                tp = pstr.tile([128, 128], F32R, tag="tr")
                nc.tensor.transpose(
                    tp[:E, :LC], xa_sb[:, c, 0:E], idr_sb[:LC, :LC]
                )
                nc.vector.tensor_copy(xT[:, c * LC:(c + 1) * LC], tp[:E, :LC])

            m_ps = psm.tile([EA, NJ, PSB], F32, tag="m")
            for c in range(NLC):
                s_ps = pss.tile([LC, NJ, PSB], F32, tag="s")
                for j in range(NJ):
                    nc.tensor.matmul(
                        s_ps[:, j, 0:NCH],
                        xT[:, c * LC:(c + 1) * LC],
                        labT[:, j * NCH:(j + 1) * NCH],
                    )
                e_sb = e_pool.tile([LC, NJ, NCH], F32R, tag="e")
                nc.scalar.activation(
                    e_sb[:], s_ps[:, :, 0:NCH],
                    mybir.ActivationFunctionType.Exp,
                    bias=bias_sb[:LC], scale=1.0,
                )
                for j in range(NJ):
                    nc.tensor.matmul(
                        m_ps[:, j, 0:NCH],
                        xa_sb[:, c, :],
                        e_sb[:, j, :],
                        start=(c == 0), stop=(c == NLC - 1),
                    )

            # out path: U^T -> transpose -> divide by Z -> DMA
            u_sb = u_pool.tile([EA, NJ, NCH], F32, tag="u")
            nc.vector.tensor_copy(u_sb[:], m_ps[:, :, 0:NCH])
            for k in range(NSP // 128):
                j, off = divmod(k * 128, NCH)
                tpo = pstr.tile([128, 128], F32, tag="tr")
                nc.tensor.transpose(
                    tpo[:, :EA], u_sb[:, j, off:off + 128], idf_sb[:EA, :EA]
                )
                rz = r_pool.tile([128, 1], F32, tag="r")
                nc.vector.reciprocal(rz[:], tpo[:, E:EA])
                o_sb = o_pool.tile([128, E], F32, tag="o")
                nc.vector.tensor_scalar_mul(o_sb[:], tpo[:, 0:E], rz[:])
                nc.sync.dma_start(
                    out=m_d[b, k * 128:(k + 1) * 128, :], in_=o_sb[:]
                )
    nc.compile()
    return nc


def _get_nc():
    if not _NC:
        _NC.append(_build())
    return _NC[0]


def kernel(x, label_feature):
    global LAST_RESULT
    x = np.ascontiguousarray(np.asarray(x, dtype=np.float32))
    lf = np.ascontiguousarray(np.asarray(label_feature, dtype=np.float32))
    assert x.shape == (B, L, E) and lf.shape == (N_TOTAL, E)

    xa = np.zeros((B, LP, EA), np.float32)
    xa[:, :L, :E] = x
    xa[:, :L, E] = 1.0
    ident = np.eye(128, dtype=np.float32)
    in_maps = []
    for r in range(NCORES):
        lo = r * NS
        hi = min(lo + NS, N_TOTAL)
        shard = np.zeros((NSP, E), np.float32)
        shard[: hi - lo] = lf[lo:hi]
        in_maps.append({"xa": xa, "lab": shard, "idr": ident, "idf": ident})

    nc = _get_nc()
    res = run_bass_kernel_spmd(
        nc, in_maps, core_ids=list(range(NCORES)), trace=TRACE
    )
    LAST_RESULT = res

    out = np.empty((B, N_TOTAL, E), np.float32)
    for r in range(NCORES):
        lo = r * NS
        hi = min(lo + NS, N_TOTAL)
        out[:, lo:hi, :] = res.results[r]["m"][:, : hi - lo, :]
    return out


# revision 3
# speedup vs baseline: 1.0367x; 1.0367x over previous
"""CodeWiseAttention kernel for Trainium2 (8 NeuronCores, label-dim sharded).

m[b,n,:] = softmax(label_feature[n] @ x[b].T) @ x[b]

Sharding: label rows N=8922 split across 8 cores (1116/core, padded to 1152);
x replicated. Per core, per batch:
  mm1 (fp32r): S^T[l,n] = xT[e,l].T @ labelT[e,n]     (xT via PE transpose)
  exp on ScalarE: expS = exp(S - 30)                   (constant shift; cancels)
  mm2 (fp32r): Uaug^T[e',n] += xa[l,e'].T @ expS^T[l,n]  accumulated over l,
      where xa has a ones column so row 100 of Uaug = Z = sum_l expS.
  out: PE-transpose Uaug^T -> [n, e'], m = U / Z, DMA out.
"""
import numpy as np
from contextlib import ExitStack

import concourse.tile as tile
from concourse import bacc, mybir
from concourse.bass_utils import run_bass_kernel_spmd

F32 = mybir.dt.float32
F32R = mybir.dt.float32r

B, L, E = 8, 2500, 100
LP = 2520          # L padded with zero rows (zero rows add nothing to U or Z)
N_TOTAL = 8922
NCORES = 8
NS = 1116          # label rows per core (core 7: 1110 real)
NSP = 1152         # padded per-core label rows
LC = 126           # l-chunk rows (even: fp32r ISA needs even innermost counts)
NLC = LP // LC     # 20 l-chunks
NCH = 384          # n-chunk width (moving dim; >=256 keeps fp32r at full rate)
NJ = NSP // NCH    # 3 n-chunks
EA = E + 1         # x augmented with ones column
PSB = 512          # psum bank stride in f32 elements
EXP_BIAS = -30.0

TRACE = False
LAST_RESULT = None

_NC = []


def _build():
    nc = bacc.Bacc("TRN2", target_bir_lowering=False, debug=False)
    xa_d = nc.dram_tensor("xa", [B, LP, EA], F32R, kind="ExternalInput").ap()
    lab_d = nc.dram_tensor("lab", [NSP, E], F32R, kind="ExternalInput").ap()
    idr_d = nc.dram_tensor("idr", [128, 128], F32R, kind="ExternalInput").ap()
    idf_d = nc.dram_tensor("idf", [128, 128], F32, kind="ExternalInput").ap()
    m_d = nc.dram_tensor("m", [B, NSP, E], F32, kind="ExternalOutput").ap()

    with tile.TileContext(nc) as tc, ExitStack() as ctx:
        consts = ctx.enter_context(tc.tile_pool(name="consts", bufs=1))
        lab_pool = ctx.enter_context(tc.tile_pool(name="labp", bufs=2))
        xa_pool = ctx.enter_context(tc.tile_pool(name="xap", bufs=2))
        xt_pool = ctx.enter_context(tc.tile_pool(name="xtp", bufs=2))
        e_pool = ctx.enter_context(tc.tile_pool(name="ep", bufs=3))
        u_pool = ctx.enter_context(tc.tile_pool(name="up", bufs=2))
        o_pool = ctx.enter_context(tc.tile_pool(name="op", bufs=4))
        r_pool = ctx.enter_context(tc.tile_pool(name="rp", bufs=4))
        pstr = ctx.enter_context(tc.tile_pool(name="pstr", bufs=2, space="PSUM"))
        pss = ctx.enter_context(tc.tile_pool(name="pss", bufs=1, space="PSUM"))
        psm = ctx.enter_context(tc.tile_pool(name="psm", bufs=1, space="PSUM"))

        idr_sb = consts.tile([128, 128], F32R)
        nc.sync.dma_start(out=idr_sb[:], in_=idr_d)
        idf_sb = consts.tile([128, 128], F32)
        nc.sync.dma_start(out=idf_sb[:], in_=idf_d)
        bias_sb = consts.tile([128, 1], F32)
        nc.vector.memset(bias_sb[:], EXP_BIAS)

        # labelT [E, NSP] via PE transposes of 128-row label chunks
        labT = consts.tile([E, NSP], F32R)
        for k in range(NSP // 128):
            lsb = lab_pool.tile([128, E], F32R, tag="lab")
            nc.sync.dma_start(out=lsb[:], in_=lab_d[k * 128:(k + 1) * 128, :])
            tp = pstr.tile([128, 128], F32R, tag="tr")
            nc.tensor.transpose(tp[:E, :], lsb[:], idr_sb[:, :])
            nc.vector.tensor_copy(labT[:, k * 128:(k + 1) * 128], tp[:E, :])

        for b in range(B):
            xa_sb = xa_pool.tile([LC, NLC, EA], F32R, tag="xa")
            nc.sync.dma_start(
                out=xa_sb[:], in_=xa_d[b].rearrange("(c p) e -> p c e", p=LC)
            )
            # xT [E, L] for this batch
            xT = xt_pool.tile([E, LP], F32R, tag="xt")
            for c in range(NLC):
                tp = pstr.tile([128, 128], F32R, tag="tr")
                nc.tensor.transpose(
                    tp[:E, :LC], xa_sb[:, c, 0:E], idr_sb[:LC, :LC]
                )
                nc.vector.tensor_copy(xT[:, c * LC:(c + 1) * LC], tp[:E, :LC])

            m_ps = psm.tile([EA, NJ, PSB], F32, tag="m")
            for c in range(NLC):
                s_ps = pss.tile([LC, NJ, PSB], F32, tag="s")
                for j in range(NJ):
                    nc.tensor.matmul(
                        s_ps[:, j, 0:NCH],
                        xT[:, c * LC:(c + 1) * LC],
                        labT[:, j * NCH:(j + 1) * NCH],
                    )
                e_sb = e_pool.tile([LC, NJ, NCH], F32R, tag="e")
                for j in range(NJ):
                    # per-j exp: bank-level deps let mm1(c+1,j) start as soon
                    # as exp(c,j) has read its bank, keeping PE dense
                    nc.scalar.activation(
                        e_sb[:, j, :], s_ps[:, j, 0:NCH],
                        mybir.ActivationFunctionType.Exp,
                        bias=bias_sb[:LC], scale=1.0,
                    )
                for j in range(NJ):
                    nc.tensor.matmul(
                        m_ps[:, j, 0:NCH],
                        xa_sb[:, c, :],
                        e_sb[:, j, :],
                        start=(c == 0), stop=(c == NLC - 1),
                    )

            # out path: U^T -> transpose -> divide by Z -> DMA
            u_sb = u_pool.tile([EA, NJ, NCH], F32, tag="u")
            nc.vector.tensor_copy(u_sb[:], m_ps[:, :, 0:NCH])
            for k in range(NSP // 128):
                j, off = divmod(k * 128, NCH)
                tpo = pstr.tile([128, 128], F32, tag="tr")
                nc.tensor.transpose(
                    tpo[:, :EA], u_sb[:, j, off:off + 128], idf_sb[:EA, :EA]
                )
                rz = r_pool.tile([128, 1], F32, tag="r")
                nc.vector.reciprocal(rz[:], tpo[:, E:EA])
                o_sb = o_pool.tile([128, E], F32, tag="o")
                nc.vector.tensor_scalar_mul(o_sb[:], tpo[:, 0:E], rz[:])
                nc.sync.dma_start(
                    out=m_d[b, k * 128:(k + 1) * 128, :], in_=o_sb[:]
                )
    nc.compile()
    return nc


def _get_nc():
    if not _NC:
        _NC.append(_build())
    return _NC[0]


def kernel(x, label_feature):
    global LAST_RESULT
    x = np.ascontiguousarray(np.asarray(x, dtype=np.float32))
    lf = np.ascontiguousarray(np.asarray(label_feature, dtype=np.float32))
    assert x.shape == (B, L, E) and lf.shape == (N_TOTAL, E)

    xa = np.zeros((B, LP, EA), np.float32)
    xa[:, :L, :E] = x
    xa[:, :L, E] = 1.0
    ident = np.eye(128, dtype=np.float32)
    in_maps = []
    for r in range(NCORES):
        lo = r * NS
        hi = min(lo + NS, N_TOTAL)
        shard = np.zeros((NSP, E), np.float32)
        shard[: hi - lo] = lf[lo:hi]
        in_maps.append({"xa": xa, "lab": shard, "idr": ident, "idf": ident})

    nc = _get_nc()
    res = run_bass_kernel_spmd(
        nc, in_maps, core_ids=list(range(NCORES)), trace=TRACE
    )
    LAST_RESULT = res

    out = np.empty((B, N_TOTAL, E), np.float32)
    for r in range(NCORES):
        lo = r * NS
        hi = min(lo + NS, N_TOTAL)
        out[:, lo:hi, :] = res.results[r]["m"][:, : hi - lo, :]
    return out
